# revision 1
# baseline (speedup 1.0000x reference)
"""AL2Loss2d Trainium2 kernel.

Reference computation:
  inputs [8, 64, 512, 512] f32, targets [8, 512, 512] int64 (values 0..18)
  - per-class sums of the 64-dim pixel features (segment_sum over 2M pixels)
  - per-class counts
  - centers = sums / max(counts, 1); pairwise cosine similarity of the 19
    centers; CosineEmbeddingLoss-style reduction to a scalar.

Strategy: data-parallel over batch. Each of the 8 NeuronCores reads one
batch element (64 MiB) and computes partial [19, 65] (sums | counts) via
one-hot accumulating matmuls on the TensorEngine:
  - host pre-permutes the shard to pixel-major layout [128, J, 64] with a
    few unused pad columns per partition (a power-of-2 HBM partition
    stride costs ~10% DMA bandwidth); targets ship as int8
  - device per tile: DMA -> fp32->fp16 convert (ScalarE) -> one-hot of
    the targets via iota+is_equal (VectorE) -> per-128-pixel-chunk matmul
    psum[19,64] += onehot[128,19].T @ x[128,64] (TensorE, fp16)
  - counts: VectorE accumulates a per-partition class histogram
    (reduce over each one-hot tile), folded across partitions by one
    final matmul against a ones vector
  - tail tiles are small so little compute remains after the last DMA
    byte lands (the kernel is HBM-bandwidth-bound at ~358 GB/s/core)
The tiny 19x19 cosine loss runs on host on the 8 gathered partials.
"""

import sys

import numpy as np

if "/opt/trn_rl_repo" not in sys.path:
    sys.path.insert(0, "/opt/trn_rl_repo")

from concourse import bacc, bass, mybir, tile  # noqa: E402
from concourse.bass_utils import run_bass_kernel_spmd  # noqa: E402

K = 19
CH = 64
CW = CH + 1  # partial layout: 64 channel sums | counts column
NCORES = 8
NPART = 128
EPS = 1e-8
XPAD = 17  # unused x j-slots per partition: keeps stride off a power of 2
TPAD = 64  # same for t (a 512 KiB stride costs ~10% HBM bandwidth)


def segments(jtot: int, g: int, tail: int):
    """Split [0, jtot) into tiles of g chunks with a tapered tail."""
    segs = []
    j = 0
    ntail = g // tail if tail else 0
    while j < jtot - ntail * tail:
        segs.append((j, g))
        j += g
    while j < jtot:
        segs.append((j, tail))
        j += tail
    assert sum(s[1] for s in segs) == jtot
    return segs


def build(jtot: int, g: int, tail: int = 0) -> bass.Bass:
    """Build the per-core Bass program (pixels = 128 * jtot)."""
    segs = segments(jtot, g, tail)
    nc = bacc.Bacc(target_bir_lowering=False, trn_type="TRN2")
    x_ext = nc.declare_dram_parameter(
        "x", [NPART, jtot + XPAD, CH], mybir.dt.float32, isOutput=False
    )
    t_ext = nc.declare_dram_parameter(
        "t", [NPART, jtot + TPAD], mybir.dt.int8, isOutput=False
    )
    out_ext = nc.declare_dram_parameter("out", [K, CW], mybir.dt.float32, isOutput=True)

    with tile.TileContext(nc) as tc:
        with (
            tc.tile_pool(name="const", bufs=1) as cpool,
            tc.tile_pool(name="xin", bufs=4) as xpool,
            tc.tile_pool(name="xh", bufs=3) as xhpool,
            tc.tile_pool(name="oh", bufs=3) as ohpool,
            tc.tile_pool(name="red", bufs=2) as redpool,
            tc.tile_pool(name="acc", bufs=1, space=bass.MemorySpace.PSUM) as psumpool,
            tc.tile_pool(name="outp", bufs=1) as opool,
        ):
            # t row = [targets (jtot) | iota 0..K-1 | zero pad]; shipping the
            # iota constants inside t keeps GpSimd instruction-free (its
            # semaphore-init + IRAM load otherwise gate the start barrier)
            t_sb = cpool.tile([NPART, jtot + K], mybir.dt.int8)
            nc.scalar.dma_start(t_sb[:], t_ext[:, : jtot + K])
            iota = t_sb[:, jtot : jtot + K]
            cnt = cpool.tile([NPART, K], mybir.dt.float32)
            nc.vector.memset(cnt[:], 0.0)
            ones16 = cpool.tile([NPART, 1], mybir.dt.float16)
            nc.vector.memset(ones16[:], 1.0)

            def onehot(oh_ap, j0, gg):
                # oh[p, k, j] = (t[p, j0+j] == k), k-major so the count
                # reduction below is over a packed (stride-1) axis
                tb = (
                    t_sb[:, j0 : j0 + gg]
                    .unsqueeze(1)
                    .broadcast_to([NPART, K, gg])
                )
                ib = iota.unsqueeze(2).broadcast_to([NPART, K, gg])
                nc.vector.tensor_tensor(
                    out=oh_ap, in0=tb, in1=ib, op=mybir.AluOpType.is_equal
                )

            def count(oh_ap):
                red = redpool.tile([NPART, K], mybir.dt.float32, tag="red")
                nc.vector.tensor_reduce(
                    red[:], oh_ap, axis=mybir.AxisListType.X, op=mybir.AluOpType.add
                )
                nc.vector.tensor_add(cnt[:], cnt[:], red[:])

            # Tail tiles depend only on t: build their one-hots and counts
            # up-front so no VectorE work remains after the last DMA byte.
            ntail = sum(gg for j0, gg in segs if gg < g)
            jt0 = jtot - ntail
            oh_tail = None
            if ntail:
                oh_tail = cpool.tile([NPART, K, ntail], mybir.dt.float16)
                onehot(oh_tail[:], jt0, ntail)
                count(oh_tail[:])

            acc = psumpool.tile([K, CH], mybir.dt.float32)
            nmm = sum(s[1] for s in segs)
            mm = 0
            for j0, gg in segs:
                xt = xpool.tile([NPART, g, CH], mybir.dt.float32, tag="xt")
                nc.sync.dma_start(xt[:, :gg, :], x_ext[:, j0 : j0 + gg, :])
                xh = xhpool.tile([NPART, g, CH], mybir.dt.float16, tag="xh")
                nc.scalar.mul(xh[:, :gg, :], xt[:, :gg, :], 1.0)
                if j0 >= jt0 and oh_tail is not None:
                    oh_mm = [oh_tail[:, :, j0 - jt0 + i] for i in range(gg)]
                else:
                    oh = ohpool.tile([NPART, K, g], mybir.dt.float16, tag="oh")
                    onehot(oh[:, :, :gg], j0, gg)
                    count(oh[:, :, :gg])
                    oh_mm = [oh[:, :, i] for i in range(gg)]
                for i in range(gg):
                    nc.tensor.matmul(
                        acc[:],
                        oh_mm[i],
                        xh[:, i, :],
                        start=(mm == 0),
                        stop=(mm == nmm - 1),
                    )
                    mm += 1
            # counts: fold the [128, K] histogram across partitions via matmul
            cnt16 = opool.tile([NPART, K], mybir.dt.float16)
            nc.vector.tensor_copy(cnt16[:], cnt[:])
            cacc = psumpool.tile([K, 1], mybir.dt.float32)
            nc.tensor.matmul(cacc[:], cnt16[:], ones16[:], start=True, stop=True)

            out_sb = opool.tile([K, CW], mybir.dt.float32)
            nc.vector.tensor_copy(out_sb[:, :CH], acc[:])
            nc.vector.tensor_copy(out_sb[:, CH : CH + 1], cacc[:])
            nc.sync.dma_start(out_ext[:], out_sb[:])
    nc.compile()
    return nc


def prep_shard(x_b: np.ndarray, t_b: np.ndarray, jtot: int):
    """x_b [64, H, W] f32, t_b [H, W] int -> device arrays.

    Pixel p*jtot + j lands at partition p, column j:
      xdev[p, j, 0:64] = features, tdev[p, j] = class id (int8)
    """
    xr = x_b.reshape(CH, NPART, jtot)
    xdev = np.empty((NPART, jtot + XPAD, CH), dtype=np.float32)
    xdev[:, :jtot, :] = xr.transpose(1, 2, 0)
    xdev[:, jtot:, :] = 0.0
    tdev = np.zeros((NPART, jtot + TPAD), dtype=np.int8)
    tdev[:, :jtot] = t_b.reshape(NPART, jtot).astype(np.int8)
    tdev[:, jtot : jtot + K] = np.arange(K, dtype=np.int8)
    return xdev, tdev


_NC_CACHE: dict = {}
TRACE = False  # set True (e.g. from test.py) to profile; result lands here
LAST_RESULT = None
G = 64
TAIL = 16


def _get_nc(jtot: int) -> bass.Bass:
    key = (jtot, G, TAIL)
    if key not in _NC_CACHE:
        _NC_CACHE[key] = build(jtot, G, TAIL)
    return _NC_CACHE[key]


def finish(partials: np.ndarray) -> np.float32:
    """partials [ncores, K, CW] -> scalar loss (host, mirrors reference)."""
    total = partials.sum(axis=0, dtype=np.float64)
    sums = total[:, :CH]
    counts = total[:, CH]
    centers = sums / np.maximum(counts, 1.0)[:, None]
    norms = np.maximum(np.sqrt((centers * centers).sum(axis=1)), EPS)
    cn = centers / norms[:, None]
    S = cn @ cn.T
    eye = np.eye(K, dtype=bool)
    per_pair = np.where(eye, 1.0 - S, np.maximum(S, 0.0))
    return np.float32(per_pair.sum() / (K * K * K))


def kernel(inputs: np.ndarray, targets: np.ndarray) -> np.ndarray:
    B, C, H, W = inputs.shape
    assert (B, C) == (NCORES, CH)
    jtot = H * W // NPART
    nc = _get_nc(jtot)

    in_maps = []
    for i in range(NCORES):
        xdev, tdev = prep_shard(np.asarray(inputs[i]), np.asarray(targets[i]), jtot)
        in_maps.append({"x": xdev, "t": tdev})

    res = run_bass_kernel_spmd(
        nc, in_maps, core_ids=list(range(NCORES)), trace=TRACE
    )
    global LAST_RESULT
    LAST_RESULT = res
    partials = np.stack([r["out"] for r in res.results])
    return np.asarray(finish(partials))



# revision 4
# speedup vs baseline: 2.6721x; 2.6721x over previous
"""AL2Loss2d Trainium2 kernel (fp8 DoubleRow edition).

Reference computation:
  inputs [8, 64, 512, 512] f32, targets [8, 512, 512] int64 (values 0..18)
  - per-class sums of the 64-dim pixel features (segment_sum over 2M pixels)
  - per-class counts
  - centers = sums / max(counts, 1); pairwise cosine similarity of the 19
    centers; CosineEmbeddingLoss-style reduction to a scalar.

Strategy: data-parallel over batch, one batch element per NeuronCore.
The rel-err budget (2e-2) is large, so the host ships features as
fp8_e4m3 (measured end-to-end rel err 5.7e-3), quartering HBM traffic
vs f32 — the kernel is DMA-bound, so this is ~4x.

Per-core layout: pixels are packed [128 partitions, 1024 pairs, 2, 65]
where the 65th column holds a per-class fp8 code mu[t] (19 distinct,
exactly-representable values). Device per tile:
  - DMA tile -> DVE builds one-hot [128, T, 2, 19] fp8 by byte-comparing
    the code column against the shipped code table (is_equal)
  - TensorE: one DoubleRow fp8 matmul per pair accumulates
    psum[19, 65] += sum_i oh[:, i, :].T @ x[:, i, :]  (256 px / instr,
    0.5 cycles/row)
  - column 64 of the accumulator is sum(mu[t]*[t==k]) = mu_k * count_k,
    so counts come out of the same matmul; host divides by mu_k.
The tiny 19x19 cosine loss runs on host on the 8 gathered partials.
"""

import sys

import ml_dtypes
import numpy as np

if "/opt/trn_rl_repo" not in sys.path:
    sys.path.insert(0, "/opt/trn_rl_repo")

from concourse import bacc, bass, mybir, tile  # noqa: E402
from concourse.bass_utils import run_bass_kernel_spmd  # noqa: E402

K = 19
CH = 64
CW = CH + 1  # 64 channel sums | mu-scaled count column
NCORES = 8
NPART = 128
EPS = 1e-8
NPAIR = 1024  # 2048 px per partition = 1024 DoubleRow pairs
PADJ = 1  # pad pair: keeps the HBM partition stride off large pow2 multiples

FP8 = ml_dtypes.float8_e4m3
# 19 distinct per-class codes, all exactly representable in e4m3 so the
# count column mu_k * count_k divides back exactly.
MU = np.array(
    [1, 2, 3, 4, 5, 6, 7, 8, 9, 10, 11, 12, 13, 14, 15, 16, 18, 20, 22],
    dtype=np.float32,
)
MU_FP8 = MU.astype(FP8)
assert np.all(MU_FP8.astype(np.float32) == MU)
MU_BYTES = MU_FP8.view(np.uint8)
assert len(set(MU_BYTES.tolist())) == K


def pair_segments(npair: int, g: int):
    """Tiles of g pairs with a tapered tail (shrinks post-DMA compute)."""
    segs = []
    j = 0
    while j < npair - g and npair - j > g:
        segs.append((j, g))
        j += g
    rem = npair - j
    for t in (g // 2, g // 4, g // 8):
        t = min(t, rem) if t > 0 else rem
        if rem > 0 and t > 0 and rem - t >= 0 and rem > g // 8:
            segs.append((j, t))
            j += t
            rem -= t
    if rem > 0:
        segs.append((j, rem))
        j += rem
    assert sum(s[1] for s in segs) == npair, segs
    return segs


def build(npair: int, g: int) -> bass.Bass:
    """Per-core Bass program (pixels = 128 * npair * 2)."""
    segs = pair_segments(npair, g)
    nc = bacc.Bacc(target_bir_lowering=False, trn_type="TRN2")
    x_ext = nc.declare_dram_parameter(
        "x", [NPART, npair + PADJ, 2, CW], mybir.dt.float8e4, isOutput=False
    )
    mu_ext = nc.declare_dram_parameter(
        "mu", [NPART, 32], mybir.dt.int8, isOutput=False
    )
    out_ext = nc.declare_dram_parameter("out", [K, CW], mybir.dt.float32, isOutput=True)

    with tile.TileContext(nc) as tc:
        with (
            tc.tile_pool(name="const", bufs=1) as cpool,
            tc.tile_pool(name="xin", bufs=3) as xpool,
            tc.tile_pool(name="oh", bufs=3) as ohpool,
            tc.tile_pool(name="acc", bufs=1, space=bass.MemorySpace.PSUM) as psumpool,
            tc.tile_pool(name="outp", bufs=1) as opool,
        ):
            # per-class code table, one byte-row per partition (tiny DMA on
            # the Act queue so it never waits behind an x tile)
            mu_sb = cpool.tile([NPART, 32], mybir.dt.int8)
            nc.scalar.dma_start(mu_sb[:], mu_ext[:])

            acc = psumpool.tile([K, CW], mybir.dt.float32)
            nmm = npair
            mm = 0
            for j0, gg in segs:
                xt = xpool.tile([NPART, g, 2, CW], mybir.dt.float8e4, tag="xt")
                nc.sync.dma_start(xt[:, :gg], x_ext[:, j0 : j0 + gg])
                # one-hot by byte equality of the fp8 class codes. Class
                # pitch is padded 19->32 because the dual-fp8 Ldweights
                # (DoubleRow) requires the k-tile step to be 16B-aligned
                # (s3_lw_dual_fp8_restrictions); cols 19..31 are never
                # written or read.
                oh = ohpool.tile([NPART, g, 2, 32], mybir.dt.float8e4, tag="oh")
                tcol = (
                    xt[:, :gg, :, CH]
                    .bitcast(mybir.dt.int8)
                    .unsqueeze(3)
                    .broadcast_to([NPART, gg, 2, K])
                )
                iob = (
                    mu_sb[:, :K]
                    .unsqueeze(1)
                    .unsqueeze(1)
                    .broadcast_to([NPART, gg, 2, K])
                )
                nc.vector.tensor_tensor(
                    out=oh[:, :gg, :, :K], in0=tcol, in1=iob, op=mybir.AluOpType.is_equal
                )
                for j in range(gg):
                    nc.tensor.matmul(
                        acc[:],
                        oh[:, j, :, :K],
                        xt[:, j],
                        start=(mm == 0),
                        stop=(mm == nmm - 1),
                        perf_mode=mybir.MatmulPerfMode.DoubleRow,
                    )
                    mm += 1
            out_sb = opool.tile([K, CW], mybir.dt.float32)
            nc.vector.tensor_copy(out_sb[:], acc[:])
            nc.sync.dma_start(out_ext[:], out_sb[:])
    nc.compile()
    return nc


def prep_shard(xq_b: np.ndarray, t_b: np.ndarray, npair: int):
    """xq_b [64, H, W] fp8, t_b [H, W] int -> device arrays."""
    npix = t_b.size
    xr = xq_b.reshape(CH, NPART, npix // NPART).transpose(1, 2, 0)
    xdev = np.zeros((NPART, npair + PADJ, 2, CW), dtype=FP8)
    xdev[:, :npair, :, :CH] = xr.reshape(NPART, npair, 2, CH)
    tcode = MU_FP8[t_b.reshape(NPART, npix // NPART)]
    xdev[:, :npair, :, CH] = tcode.reshape(NPART, npair, 2)
    mudev = np.zeros((NPART, 32), dtype=np.int8)
    mudev[:, :K] = MU_BYTES.view(np.int8)
    return xdev, mudev


_NC_CACHE: dict = {}
TRACE = False  # set True (e.g. from test.py) to profile; result lands here
LAST_RESULT = None
G = 128  # pairs per tile


def _get_nc(npair: int) -> bass.Bass:
    key = (npair, G)
    if key not in _NC_CACHE:
        _NC_CACHE[key] = build(npair, G)
    return _NC_CACHE[key]


def finish(partials: np.ndarray) -> np.float32:
    """partials [ncores, K, CW] -> scalar loss (host, mirrors reference)."""
    total = partials.sum(axis=0, dtype=np.float64)
    sums = total[:, :CH]
    counts = total[:, CH] / MU.astype(np.float64)
    centers = sums / np.maximum(counts, 1.0)[:, None]
    norms = np.maximum(np.sqrt((centers * centers).sum(axis=1)), EPS)
    cn = centers / norms[:, None]
    S = cn @ cn.T
    eye = np.eye(K, dtype=bool)
    per_pair = np.where(eye, 1.0 - S, np.maximum(S, 0.0))
    return np.float32(per_pair.sum() / (K * K * K))


def kernel(inputs: np.ndarray, targets: np.ndarray) -> np.ndarray:
    B, C, H, W = inputs.shape
    assert (B, C) == (NCORES, CH)
    npair = H * W // NPART // 2
    nc = _get_nc(npair)

    xq = np.asarray(inputs).astype(FP8)
    tgt = np.asarray(targets)
    in_maps = []
    for i in range(NCORES):
        xdev, mudev = prep_shard(xq[i], tgt[i], npair)
        in_maps.append({"x": xdev, "mu": mudev})

    res = run_bass_kernel_spmd(
        nc, in_maps, core_ids=list(range(NCORES)), trace=TRACE
    )
    global LAST_RESULT
    LAST_RESULT = res
    partials = np.stack([r["out"] for r in res.results])
    return np.asarray(finish(partials))
